# revision 31
# baseline (speedup 1.0000x reference)
"""Trainium2 Bass kernel for the NP/NY/NU RNN scan (nn_BlackBoxModel_24489903521937).

Model (per step t, batch row b):
    x_t   = [y_t, y_{t-4..t-1}, u_{t-4..t-1}, u_t]          (60)
    h1    = tanh(x_t @ W1 + b1)                              (128)
    h2    = tanh(h1 @ W2 + b2)                               (128)
    y_{t+1} = h2 @ W3 + b3                                   (8)
    output ys[:, t] = y_t

Strategy (pure data parallel, batch 4096 -> 8 cores x 512):
  * feature-major layout: features on SBUF partitions, batch on the free dim.
  * y-history lives in 4 ring slots of a [128, B] staging tile, one slot per
    32-partition strip.  The x @ W1 product is: one K=128 matmul against
    phase-permuted W1 blocks (C_p, p = t mod 4), one K=32 matmul against the
    raw 5-step u window (B rows, a fresh DMA'd tile per step so the u path
    has no write-after-read coupling to the staging tile and prefetches
    deep), and a composed (W3 @ A0) matmul from h2 directly, so the
    recurrent cycle is just tanh -> mm(W2) -> tanh -> mm(W3 A0).
  * CHUNKS=2 column chunks software-pipeline the recurrent chain: while
    chunk 0 is in an activation, chunk 1's matmul runs, keeping ScalarE
    (the bottleneck engine) nearly fully busy.  Each chunk owns private
    full PSUM banks (PSUM bank read/write collisions are fatal), including
    per-chunk mm3/staging-write chains so each chunk's y lands a full step
    before mmX(t+2) reads it.
  * outputs retire from the staging tile by raw feature-major DMA every 4
    steps; the host does the final [T,8,B] -> [B,T,8] transpose.  u-window
    DMAs ride the GpSimd engine queue so the flush DMAs' long semaphore
    waits on the Sync queue cannot head-of-line block them.
  * a warm-up burst of back-to-back matmuls at kernel start engages the PE
    HAM clock gate (K=8/8, 2.4 GHz); the tight steady-state schedule keeps
    it warm.
  * matmul operands are fp16 (1 cycle/row, fp32 PSUM accumulate); the
    5-step fading memory of the state keeps fp16 error flat (~6e-4).
"""

import numpy as np

NP_, NY, NU = 4, 8, 4
B, T, H = 4096, 256, 128
NCORES = 8
BC = B // NCORES  # 512 batch rows per core
CHUNKS = 2        # column chunks for the critical tanh/matmul cycle
CW = BC // CHUNKS
PF = 8            # u-window DMA prefetch depth (steps ahead)
NSLOT = 4         # y ring slots (one per 32-partition strip)
HEAT = 8          # warm-up matmuls at kernel start

_COMPILED = {}


def _build_program():
    import concourse.mybir as mybir
    import concourse.tile as tile
    from concourse import bacc

    f32 = mybir.dt.float32
    fh = mybir.dt.float16
    Tanh = mybir.ActivationFunctionType.Tanh

    nc = bacc.Bacc("TRN2", target_bir_lowering=False, debug=False)

    d_stag0 = nc.dram_tensor("stag0", [128, BC], fh, kind="ExternalInput")
    d_uwin = nc.dram_tensor("uwin", [T, 32, BC], fh, kind="ExternalInput")
    # 8 C matrices: [0..3] steady phases (t % 4), [4..7] boot steps t=0..3
    d_cmats = nc.dram_tensor("cmats", [128, 8 * 128], fh, kind="ExternalInput")
    d_bmat = nc.dram_tensor("bmat", [32, 128], fh, kind="ExternalInput")
    d_w2 = nc.dram_tensor("w2", [128, 128], fh, kind="ExternalInput")
    d_wc = nc.dram_tensor("wc", [128, 128], fh, kind="ExternalInput")
    d_w3 = nc.dram_tensor("w3", [128, 8], fh, kind="ExternalInput")
    d_b1 = nc.dram_tensor("b1v", [128, 1], f32, kind="ExternalInput")
    d_b1b = nc.dram_tensor("b1b", [128, 1], f32, kind="ExternalInput")
    d_b2 = nc.dram_tensor("b2v", [128, 1], f32, kind="ExternalInput")
    d_b3 = nc.dram_tensor("b3v", [8, 1], f32, kind="ExternalInput")
    d_out2 = nc.dram_tensor("out2", [T // 4, 4, 8, BC], fh, kind="ExternalOutput")

    with tile.TileContext(nc) as tc:
        with (
            tc.tile_pool(name="const", bufs=1) as cpool,
            tc.tile_pool(name="stagp", bufs=1) as spool,
            tc.tile_pool(name="upool", bufs=12) as upool,
            tc.tile_pool(name="hpool", bufs=2) as hpool,
            tc.tile_pool(name="ph1a", bufs=2, space="PSUM") as ph1a,
            tc.tile_pool(name="ph1b", bufs=2, space="PSUM") as ph1b,
            tc.tile_pool(name="ph2a", bufs=1, space="PSUM") as ph2a,
            tc.tile_pool(name="ph2b", bufs=1, space="PSUM") as ph2b,
            tc.tile_pool(name="pypa", bufs=1, space="PSUM") as pypa,
            tc.tile_pool(name="pypb", bufs=1, space="PSUM") as pypb,
        ):
            t_cm = cpool.tile_from(d_cmats[:])

            # --- PE warm-up: dense back-to-back matmuls engage the HAM
            #     clock gate (K=8/8, 2.4 GHz) and cover the initial DMAs.
            #     Reuses the pypa bank (same tag): strictly PE-ordered before
            #     any step-0 use, so no PSUM collision is possible. ---
            pyp_pools = [pypa, pypb]
            heat = pypa.tile([128, BC], f32, name="heat", tag="yp")
            for _ in range(HEAT):
                nc.tensor.matmul(heat[:, :], t_cm[:, 0:128], t_cm[:, 0:BC])

            t_bm = cpool.tile_from(d_bmat[:])
            t_w2 = cpool.tile_from(d_w2[:])
            t_wc = cpool.tile_from(d_wc[:])
            t_w3 = cpool.tile_from(d_w3[:])
            t_b1 = cpool.tile_from(d_b1[:])
            t_b1b = cpool.tile_from(d_b1b[:])
            t_b2 = cpool.tile_from(d_b2[:])
            t_b3 = cpool.tile_from(d_b3[:])

            stag = spool.tile([128, BC], fh, name="stag")
            nc.sync.dma_start(stag[:], d_stag0[:])

            def cmat(i):
                return t_cm[:, 128 * i:128 * i + 128]

            ph1_pools = [ph1a, ph1b]
            ph2_pools = [ph2a, ph2b]
            utiles = {}

            def prefetch_u(tt):
                # GpSimd-engine queue: keeps these triggers off the Sync
                # queue, where the flush DMAs' long semaphore waits would
                # block them (head-of-line) and land the u windows late.
                ut = upool.tile([32, BC], fh, name="uw", tag="uw")
                nc.gpsimd.dma_start(ut[:], d_uwin[tt])
                utiles[tt] = ut

            for tt in range(PF):
                prefetch_u(tt)

            def emit_mmx(tt, ph1_t):
                """y-history + u-window matmuls for step tt (chunked).

                Opens the ph1(tt) accumulation group; mmC of step tt-1
                (emitted later, executed later) closes it.
                """
                cidx = 4 + tt if tt < 4 else tt % NSLOT
                ut = utiles.pop(tt)
                for c in range(CHUNKS):
                    cs = slice(c * CW, (c + 1) * CW)
                    nc.tensor.matmul(
                        ph1_t[c][:, 0:CW],
                        cmat(cidx),
                        stag[:, cs],
                        start=True, stop=False, skip_group_check=True,
                    )
                    nc.tensor.matmul(
                        ph1_t[c][:, 0:CW],
                        t_bm[:, :],
                        ut[:, cs],
                        start=False, stop=(tt == 0), skip_group_check=True,
                    )

            def flush(t0):
                """Export y_{t0..t0+3} (all 4 slots) feature-major to DRAM;
                the host transposes to batch-major at the end.

                Emitted at step t0+3 BEFORE that step's staging write, so slot
                (t0+4)%4 still holds y_{t0}.
                """
                for s in range(4):
                    nc.sync.dma_start(
                        d_out2[t0 // 4, s], stag[32 * s:32 * s + 8, :]
                    )

            ph1_cur = [ph1_pools[c].tile([128, BC], f32, name="h1p", tag="h1p")
                       for c in range(CHUNKS)]
            emit_mmx(0, ph1_cur)

            for t in range(T):
                # --- tanh1 ---
                h1_t = hpool.tile([128, BC], fh, name="h1", tag="h1")
                bias1 = t_b1b if t == 0 else t_b1
                for c in range(CHUNKS):
                    cs = slice(c * CW, (c + 1) * CW)
                    nc.scalar.activation(
                        h1_t[:, cs], ph1_cur[c][:, 0:CW], Tanh, bias=bias1[:, 0:1]
                    )

                # --- pre-issue next step's x-side matmuls (fill the PE while
                #     the activations run; must precede this step's staging
                #     write so the stale y_{t-3} read stays dependency-free) ---
                ph1_next = None
                if t + 1 < T:
                    ph1_next = [
                        ph1_pools[c].tile([128, BC], f32, name="h1p", tag="h1p")
                        for c in range(CHUNKS)
                    ]
                    emit_mmx(t + 1, ph1_next)

                # --- mm2 ---
                ph2_t = [ph2_pools[c].tile([128, BC], f32, name="h2p", tag="h2p")
                         for c in range(CHUNKS)]
                with tc.high_priority():
                    for c in range(CHUNKS):
                        cs = slice(c * CW, (c + 1) * CW)
                        nc.tensor.matmul(
                            ph2_t[c][:, 0:CW],
                            t_w2[:, :],
                            h1_t[:, cs],
                        )

                # --- tanh2 ---
                h2_t = hpool.tile([128, BC], fh, name="h2", tag="h2")
                for c in range(CHUNKS):
                    cs = slice(c * CW, (c + 1) * CW)
                    nc.scalar.activation(
                        h2_t[:, cs], ph2_t[c][:, 0:CW], Tanh, bias=t_b2[:, 0:1]
                    )

                # --- mmC: ph1(t+1) += (W3 A0)^T h2_t  (closes the group) ---
                if t + 1 < T:
                    with tc.high_priority():
                        for c in range(CHUNKS):
                            cs = slice(c * CW, (c + 1) * CW)
                            nc.tensor.matmul(
                                ph1_next[c][:, 0:CW],
                                t_wc[:, :],
                                h2_t[:, cs],
                                start=False, stop=True,
                                skip_group_check=True,
                            )

                # --- output flush (before this step's staging write) ---
                if t % 4 == 3:
                    flush(t - 3)

                # --- mm3 + staging write (y_{t+1} = W3^T h2 + b3), chunked
                #     so each chunk's y lands a full step before mmX(t+2)
                #     reads it and stays off the recurrent critical path ---
                if t < T - 1:
                    s_new = (t + 1) % NSLOT
                    for c in range(CHUNKS):
                        cs = slice(c * CW, (c + 1) * CW)
                        pyp_t = pyp_pools[c].tile(
                            [128, BC], f32, name="yp", tag="yp")
                        nc.tensor.matmul(
                            pyp_t[0:8, 0:CW], t_w3[:, :], h2_t[:, cs]
                        )
                        nc.vector.tensor_scalar_add(
                            stag[32 * s_new:32 * s_new + 8, cs],
                            pyp_t[0:8, 0:CW], t_b3[:, 0:1]
                        )

                if t + PF < T:
                    prefetch_u(t + PF)

                ph1_cur = ph1_next

    nc.compile()
    return nc


def _host_prep(useq, yz0, W1, b1, W2, b2, W3, b3):
    """Build the per-core input maps (all host-side numpy)."""
    useq = np.ascontiguousarray(useq, dtype=np.float32)
    yz0 = np.ascontiguousarray(yz0, dtype=np.float32)
    W1 = np.asarray(W1, dtype=np.float32)
    W2 = np.ascontiguousarray(W2, dtype=np.float32)
    W3 = np.ascontiguousarray(W3, dtype=np.float32)
    b1 = np.asarray(b1, dtype=np.float32)
    b2 = np.asarray(b2, dtype=np.float32)
    b3 = np.asarray(b3, dtype=np.float32)

    A = {0: W1[0:8], 4: W1[8:16], 3: W1[16:24], 2: W1[24:32], 1: W1[32:40]}
    Bstack = W1[40:60]  # u_{t-4..t} stacked chronologically

    # staging rows: slot s -> [32s, 32s+8) holds the y ring;
    #               boot block s -> [32s+8, 32s+16) holds y_{-(s+1)}
    cmats = np.zeros((8, 128, 128), dtype=np.float32)
    for p in range(NSLOT):  # steady phases, t >= 4: every slot one A_k
        for s in range(NSLOT):
            k = ((p - s - 1) % 4) + 1
            cmats[p, 32 * s:32 * s + 8] = A[k]
    for tt in range(4):  # boot steps t=0..3
        cb = cmats[4 + tt]
        for k in range(1, 5):
            if tt - k >= 0:
                s = (tt - k) % 4
                cb[32 * s:32 * s + 8] += A[k]
            else:
                s = k - tt - 1
                cb[32 * s + 8:32 * s + 16] += A[k]
        if tt == 0:
            cb[0:8] += A[0]  # slot 0 carries y_0 directly at t=0
    cmats2d = np.ascontiguousarray(
        cmats.transpose(1, 0, 2).reshape(128, 8 * 128)
    )

    bmat = np.zeros((32, 128), dtype=np.float32)
    bmat[0:20] = Bstack

    WC = np.ascontiguousarray(W3 @ A[0])          # [128, 128]
    b1_eff = (b1 + A[0].T @ b3).reshape(128, 1)   # mmC path lacks A0^T b3
    b1_boot = b1.reshape(128, 1)
    b2v = b2.reshape(128, 1)
    b3v = b3.reshape(8, 1)

    cmats_fh = cmats2d.astype(np.float16)
    bmat_fh = bmat.astype(np.float16)
    w2_fh = W2.astype(np.float16)
    wc_fh = WC.astype(np.float16)
    w3_fh = W3.astype(np.float16)

    in_maps = []
    for c in range(NCORES):
        bs = slice(c * BC, (c + 1) * BC)
        u_c = useq[bs]      # [BC, T, 4]
        yz_c = yz0[bs]      # [BC, 56]

        stag0 = np.zeros((128, BC), dtype=np.float32)
        stag0[0:8] = yz_c[:, 0:8].T               # slot 0 = y_0
        for s in range(4):                         # boot blocks y_{-(s+1)}
            blk = yz_c[:, 8 + 8 * (3 - s):16 + 8 * (3 - s)]  # ypseq newest last
            stag0[32 * s + 8:32 * s + 16] = blk.T

        # sliding u-windows for the K=32 u matmul (rows 20..31 zero)
        uhist = yz_c[:, 40:56].reshape(BC, 4, 4)          # u_{-4..-1}
        uext = np.concatenate([uhist, u_c], axis=1)       # [BC, T+4, 4]
        sw = np.lib.stride_tricks.sliding_window_view(uext, 5, axis=1)
        # sw: [BC, T, 4, 5] -> uwin [T, 20, BC] (chronological rows)
        uwin = np.zeros((T, 32, BC), dtype=np.float16)
        uwin[:, 0:20] = sw.transpose(1, 3, 2, 0).reshape(T, 20, BC)

        in_maps.append({
            "stag0": stag0.astype(np.float16),
            "uwin": uwin,
            "cmats": cmats_fh,
            "bmat": bmat_fh,
            "w2": w2_fh,
            "wc": wc_fh,
            "w3": w3_fh,
            "b1v": np.ascontiguousarray(b1_eff),
            "b1b": np.ascontiguousarray(b1_boot),
            "b2v": np.ascontiguousarray(b2v),
            "b3v": np.ascontiguousarray(b3v),
        })
    return in_maps


def get_program():
    if "nc" not in _COMPILED:
        _enable_ldw_opt()
        _COMPILED["nc"] = _build_program()
    return _COMPILED["nc"]


def _enable_ldw_opt():
    """Allow walrus to double-buffer LDWEIGHTS (background weight loads).

    The environment default is --enable-ldw-opt=false, which serializes
    every LDWEIGHTS behind the previous matmul's drain; with ~9 weight
    switches per RNN step that costs ~2x on the tensor engine.
    """
    try:
        from concourse.compiler_utils import get_compiler_flags, set_compiler_flags

        flags = get_compiler_flags()
        new = [f.replace("--enable-ldw-opt=false", "--enable-ldw-opt=true") for f in flags]
        if new != flags:
            set_compiler_flags(new)
    except Exception:
        pass


def run_cores(in_maps, **kwargs):
    from concourse.bass_utils import run_bass_kernel_spmd

    _enable_ldw_opt()
    nc = get_program()
    return run_bass_kernel_spmd(nc, in_maps, core_ids=list(range(NCORES)), **kwargs)


def assemble(res):
    outs = []
    for r in res.results:
        buf = np.asarray(r["out2"], dtype=np.float32)   # [T/4, 4, 8, BC]
        ys = buf.transpose(3, 0, 1, 2).reshape(BC, T, NY)
        outs.append(ys)
    return np.concatenate(outs, axis=0)


def kernel(useq, yz0, W1, b1, W2, b2, W3, b3):
    in_maps = _host_prep(useq, yz0, W1, b1, W2, b2, W3, b3)
    res = run_cores(in_maps)
    return assemble(res)


# revision 33
# speedup vs baseline: 1.4701x; 1.4701x over previous
"""Trainium2 Bass kernel for the NP/NY/NU RNN scan (nn_BlackBoxModel_24489903521937).

Model (per step t, batch row b):
    x_t   = [y_t, y_{t-4..t-1}, u_{t-4..t-1}, u_t]          (60)
    h1    = tanh(x_t @ W1 + b1)                              (128)
    h2    = tanh(h1 @ W2 + b2)                               (128)
    y_{t+1} = h2 @ W3 + b3                                   (8)
    output ys[:, t] = y_t

Strategy (pure data parallel, batch 4096 -> 8 cores x 512):
  * feature-major layout: features on SBUF partitions, batch on the free dim.
  * y-history lives in 4 ring slots of a [128, B] staging tile, one slot per
    32-partition strip.  The x @ W1 product is: one K=128 matmul against
    phase-permuted W1 blocks (C_p, p = t mod 4), one K=32 matmul against the
    raw 5-step u window (B rows, a fresh DMA'd tile per step so the u path
    has no write-after-read coupling to the staging tile and prefetches
    deep), and a composed (W3 @ A0) matmul from h2 directly, so the
    recurrent cycle is just tanh -> mm(W2) -> tanh -> mm(W3 A0).
  * CHUNKS=2 column chunks software-pipeline the recurrent chain: while
    chunk 0 is in an activation, chunk 1's matmul runs, keeping ScalarE
    (the bottleneck engine) nearly fully busy.  Each chunk owns private
    full PSUM banks (PSUM bank read/write collisions are fatal), including
    per-chunk mm3/staging-write chains so each chunk's y lands a full step
    before mmX(t+2) reads it.
  * outputs retire from the staging tile by raw feature-major DMA every 4
    steps; the host does the final [T,8,B] -> [B,T,8] transpose.  u-window
    DMAs ride the GpSimd engine queue so the flush DMAs' long semaphore
    waits on the Sync queue cannot head-of-line block them.
  * a warm-up burst of back-to-back matmuls at kernel start engages the PE
    HAM clock gate (K=8/8, 2.4 GHz); the tight steady-state schedule keeps
    it warm.
  * matmul operands are fp16 (1 cycle/row, fp32 PSUM accumulate); the
    5-step fading memory of the state keeps fp16 error flat (~6e-4).
"""

import numpy as np

NP_, NY, NU = 4, 8, 4
B, T, H = 4096, 256, 128
NCORES = 8
BC = B // NCORES  # 512 batch rows per core
CHUNKS = 2        # column chunks for the critical tanh/matmul cycle
CW = BC // CHUNKS
PF = 6            # u-window DMA prefetch depth (steps ahead)
NSLOT = 4         # y ring slots (one per 32-partition strip)
HEAT = 14         # warm-up matmuls at kernel start

_COMPILED = {}


def _build_program():
    import concourse.mybir as mybir
    import concourse.tile as tile
    from concourse import bacc

    f32 = mybir.dt.float32
    fh = mybir.dt.float16
    Tanh = mybir.ActivationFunctionType.Tanh

    nc = bacc.Bacc("TRN2", target_bir_lowering=False, debug=False)

    d_stag0 = nc.dram_tensor("stag0", [128, BC], fh, kind="ExternalInput")
    d_uwin = nc.dram_tensor("uwin", [T, 32, BC], fh, kind="ExternalInput")
    # 8 C matrices: [0..3] steady phases (t % 4), [4..7] boot steps t=0..3
    d_cmats = nc.dram_tensor("cmats", [128, 8 * 128], fh, kind="ExternalInput")
    d_bmat = nc.dram_tensor("bmat", [32, 128], fh, kind="ExternalInput")
    d_w2 = nc.dram_tensor("w2", [128, 128], fh, kind="ExternalInput")
    d_wc = nc.dram_tensor("wc", [128, 128], fh, kind="ExternalInput")
    d_w3 = nc.dram_tensor("w3", [128, 8], fh, kind="ExternalInput")
    d_b1 = nc.dram_tensor("b1v", [128, 1], f32, kind="ExternalInput")
    d_b1b = nc.dram_tensor("b1b", [128, 1], f32, kind="ExternalInput")
    d_b2 = nc.dram_tensor("b2v", [128, 1], f32, kind="ExternalInput")
    d_b3 = nc.dram_tensor("b3v", [8, 1], f32, kind="ExternalInput")
    d_out2 = nc.dram_tensor("out2", [T // 4, 4, 8, BC], fh, kind="ExternalOutput")

    with tile.TileContext(nc) as tc:
        with (
            tc.tile_pool(name="const", bufs=1) as cpool,
            tc.tile_pool(name="stagp", bufs=1) as spool,
            tc.tile_pool(name="upool", bufs=8) as upool,
            tc.tile_pool(name="hpool", bufs=2) as hpool,
            tc.tile_pool(name="ph1a", bufs=2, space="PSUM") as ph1a,
            tc.tile_pool(name="ph1b", bufs=2, space="PSUM") as ph1b,
            tc.tile_pool(name="ph2a", bufs=1, space="PSUM") as ph2a,
            tc.tile_pool(name="ph2b", bufs=1, space="PSUM") as ph2b,
            tc.tile_pool(name="pypa", bufs=1, space="PSUM") as pypa,
            tc.tile_pool(name="pypb", bufs=1, space="PSUM") as pypb,
        ):
            t_cm = cpool.tile_from(d_cmats[:])

            # --- PE warm-up: dense back-to-back matmuls engage the HAM
            #     clock gate (K=8/8, 2.4 GHz) and cover the initial DMAs.
            #     Reuses the pypa bank (same tag): strictly PE-ordered before
            #     any step-0 use, so no PSUM collision is possible. ---
            pyp_pools = [pypa, pypb]
            heat = pypa.tile([128, BC], f32, name="heat", tag="yp")
            for _ in range(HEAT):
                nc.tensor.matmul(heat[:, :], t_cm[:, 0:128], t_cm[:, 0:BC])

            t_bm = cpool.tile_from(d_bmat[:])
            t_w2 = cpool.tile_from(d_w2[:])
            t_wc = cpool.tile_from(d_wc[:])
            t_w3 = cpool.tile_from(d_w3[:])
            t_b1 = cpool.tile_from(d_b1[:])
            t_b1b = cpool.tile_from(d_b1b[:])
            t_b2 = cpool.tile_from(d_b2[:])
            t_b3 = cpool.tile_from(d_b3[:])

            stag = spool.tile([128, BC], fh, name="stag")
            nc.sync.dma_start(stag[:], d_stag0[:])

            def cmat(i):
                return t_cm[:, 128 * i:128 * i + 128]

            ph1_pools = [ph1a, ph1b]
            ph2_pools = [ph2a, ph2b]
            utiles = {}

            def prefetch_u(tt):
                # GpSimd-engine queue: keeps these triggers off the Sync
                # queue, where the flush DMAs' long semaphore waits would
                # block them (head-of-line) and land the u windows late.
                ut = upool.tile([32, BC], fh, name="uw", tag="uw")
                nc.gpsimd.dma_start(ut[:], d_uwin[tt])
                utiles[tt] = ut

            for tt in range(PF):
                prefetch_u(tt)

            def emit_mmx(tt, ph1_t):
                """y-history + u-window matmuls for step tt (chunked).

                Opens the ph1(tt) accumulation group; mmC of step tt-1
                (emitted later, executed later) closes it.
                """
                cidx = 4 + tt if tt < 4 else tt % NSLOT
                ut = utiles.pop(tt)
                for c in range(CHUNKS):
                    cs = slice(c * CW, (c + 1) * CW)
                    nc.tensor.matmul(
                        ph1_t[c][:, 0:CW],
                        cmat(cidx),
                        stag[:, cs],
                        start=True, stop=False, skip_group_check=True,
                    )
                    nc.tensor.matmul(
                        ph1_t[c][:, 0:CW],
                        t_bm[:, :],
                        ut[:, cs],
                        start=False, stop=(tt == 0), skip_group_check=True,
                    )

            def flush(t0):
                """Export y_{t0..t0+3} (all 4 slots) feature-major to DRAM;
                the host transposes to batch-major at the end.

                Emitted at step t0+3 BEFORE that step's staging write, so slot
                (t0+4)%4 still holds y_{t0}.
                """
                for s in range(4):
                    nc.sync.dma_start(
                        d_out2[t0 // 4, s], stag[32 * s:32 * s + 8, :]
                    )

            ph1_cur = [ph1_pools[c].tile([128, BC], f32, name="h1p", tag="h1p")
                       for c in range(CHUNKS)]
            emit_mmx(0, ph1_cur)

            for t in range(T):
                # --- tanh1 ---
                h1_t = hpool.tile([128, BC], fh, name="h1", tag="h1")
                bias1 = t_b1b if t == 0 else t_b1
                for c in range(CHUNKS):
                    cs = slice(c * CW, (c + 1) * CW)
                    nc.scalar.activation(
                        h1_t[:, cs], ph1_cur[c][:, 0:CW], Tanh, bias=bias1[:, 0:1]
                    )

                # --- pre-issue next step's x-side matmuls (fill the PE while
                #     the activations run; must precede this step's staging
                #     write so the stale y_{t-3} read stays dependency-free) ---
                ph1_next = None
                if t + 1 < T:
                    ph1_next = [
                        ph1_pools[c].tile([128, BC], f32, name="h1p", tag="h1p")
                        for c in range(CHUNKS)
                    ]
                    emit_mmx(t + 1, ph1_next)

                # --- mm2 ---
                ph2_t = [ph2_pools[c].tile([128, BC], f32, name="h2p", tag="h2p")
                         for c in range(CHUNKS)]
                with tc.high_priority():
                    for c in range(CHUNKS):
                        cs = slice(c * CW, (c + 1) * CW)
                        nc.tensor.matmul(
                            ph2_t[c][:, 0:CW],
                            t_w2[:, :],
                            h1_t[:, cs],
                        )

                # --- tanh2 ---
                h2_t = hpool.tile([128, BC], fh, name="h2", tag="h2")
                for c in range(CHUNKS):
                    cs = slice(c * CW, (c + 1) * CW)
                    nc.scalar.activation(
                        h2_t[:, cs], ph2_t[c][:, 0:CW], Tanh, bias=t_b2[:, 0:1]
                    )

                # --- mmC: ph1(t+1) += (W3 A0)^T h2_t  (closes the group) ---
                if t + 1 < T:
                    with tc.high_priority():
                        for c in range(CHUNKS):
                            cs = slice(c * CW, (c + 1) * CW)
                            nc.tensor.matmul(
                                ph1_next[c][:, 0:CW],
                                t_wc[:, :],
                                h2_t[:, cs],
                                start=False, stop=True,
                                skip_group_check=True,
                            )

                # --- output flush (before this step's staging write) ---
                if t % 4 == 3:
                    flush(t - 3)

                # --- mm3 + staging write (y_{t+1} = W3^T h2 + b3), chunked
                #     so each chunk's y lands a full step before mmX(t+2)
                #     reads it and stays off the recurrent critical path ---
                if t < T - 1:
                    s_new = (t + 1) % NSLOT
                    for c in range(CHUNKS):
                        cs = slice(c * CW, (c + 1) * CW)
                        pyp_t = pyp_pools[c].tile(
                            [128, BC], f32, name="yp", tag="yp")
                        nc.tensor.matmul(
                            pyp_t[0:8, 0:CW], t_w3[:, :], h2_t[:, cs]
                        )
                        nc.vector.tensor_scalar_add(
                            stag[32 * s_new:32 * s_new + 8, cs],
                            pyp_t[0:8, 0:CW], t_b3[:, 0:1]
                        )

                if t + PF < T:
                    prefetch_u(t + PF)

                ph1_cur = ph1_next

    nc.compile()
    return nc


def _host_prep(useq, yz0, W1, b1, W2, b2, W3, b3):
    """Build the per-core input maps (all host-side numpy)."""
    useq = np.ascontiguousarray(useq, dtype=np.float32)
    yz0 = np.ascontiguousarray(yz0, dtype=np.float32)
    W1 = np.asarray(W1, dtype=np.float32)
    W2 = np.ascontiguousarray(W2, dtype=np.float32)
    W3 = np.ascontiguousarray(W3, dtype=np.float32)
    b1 = np.asarray(b1, dtype=np.float32)
    b2 = np.asarray(b2, dtype=np.float32)
    b3 = np.asarray(b3, dtype=np.float32)

    A = {0: W1[0:8], 4: W1[8:16], 3: W1[16:24], 2: W1[24:32], 1: W1[32:40]}
    Bstack = W1[40:60]  # u_{t-4..t} stacked chronologically

    # staging rows: slot s -> [32s, 32s+8) holds the y ring;
    #               boot block s -> [32s+8, 32s+16) holds y_{-(s+1)}
    cmats = np.zeros((8, 128, 128), dtype=np.float32)
    for p in range(NSLOT):  # steady phases, t >= 4: every slot one A_k
        for s in range(NSLOT):
            k = ((p - s - 1) % 4) + 1
            cmats[p, 32 * s:32 * s + 8] = A[k]
    for tt in range(4):  # boot steps t=0..3
        cb = cmats[4 + tt]
        for k in range(1, 5):
            if tt - k >= 0:
                s = (tt - k) % 4
                cb[32 * s:32 * s + 8] += A[k]
            else:
                s = k - tt - 1
                cb[32 * s + 8:32 * s + 16] += A[k]
        if tt == 0:
            cb[0:8] += A[0]  # slot 0 carries y_0 directly at t=0
    cmats2d = np.ascontiguousarray(
        cmats.transpose(1, 0, 2).reshape(128, 8 * 128)
    )

    bmat = np.zeros((32, 128), dtype=np.float32)
    bmat[0:20] = Bstack

    WC = np.ascontiguousarray(W3 @ A[0])          # [128, 128]
    b1_eff = (b1 + A[0].T @ b3).reshape(128, 1)   # mmC path lacks A0^T b3
    b1_boot = b1.reshape(128, 1)
    b2v = b2.reshape(128, 1)
    b3v = b3.reshape(8, 1)

    cmats_fh = cmats2d.astype(np.float16)
    bmat_fh = bmat.astype(np.float16)
    w2_fh = W2.astype(np.float16)
    wc_fh = WC.astype(np.float16)
    w3_fh = W3.astype(np.float16)

    in_maps = []
    for c in range(NCORES):
        bs = slice(c * BC, (c + 1) * BC)
        u_c = useq[bs]      # [BC, T, 4]
        yz_c = yz0[bs]      # [BC, 56]

        stag0 = np.zeros((128, BC), dtype=np.float32)
        stag0[0:8] = yz_c[:, 0:8].T               # slot 0 = y_0
        for s in range(4):                         # boot blocks y_{-(s+1)}
            blk = yz_c[:, 8 + 8 * (3 - s):16 + 8 * (3 - s)]  # ypseq newest last
            stag0[32 * s + 8:32 * s + 16] = blk.T

        # sliding u-windows for the K=32 u matmul (rows 20..31 zero)
        uhist = yz_c[:, 40:56].reshape(BC, 4, 4)          # u_{-4..-1}
        uext = np.concatenate([uhist, u_c], axis=1)       # [BC, T+4, 4]
        sw = np.lib.stride_tricks.sliding_window_view(uext, 5, axis=1)
        # sw: [BC, T, 4, 5] -> uwin [T, 20, BC] (chronological rows)
        uwin = np.zeros((T, 32, BC), dtype=np.float16)
        uwin[:, 0:20] = sw.transpose(1, 3, 2, 0).reshape(T, 20, BC)

        in_maps.append({
            "stag0": stag0.astype(np.float16),
            "uwin": uwin,
            "cmats": cmats_fh,
            "bmat": bmat_fh,
            "w2": w2_fh,
            "wc": wc_fh,
            "w3": w3_fh,
            "b1v": np.ascontiguousarray(b1_eff),
            "b1b": np.ascontiguousarray(b1_boot),
            "b2v": np.ascontiguousarray(b2v),
            "b3v": np.ascontiguousarray(b3v),
        })
    return in_maps


def get_program():
    if "nc" not in _COMPILED:
        _enable_ldw_opt()
        _COMPILED["nc"] = _build_program()
    return _COMPILED["nc"]


def _enable_ldw_opt():
    """Allow walrus to double-buffer LDWEIGHTS (background weight loads).

    The environment default is --enable-ldw-opt=false, which serializes
    every LDWEIGHTS behind the previous matmul's drain; with ~9 weight
    switches per RNN step that costs ~2x on the tensor engine.
    """
    try:
        from concourse.compiler_utils import get_compiler_flags, set_compiler_flags

        flags = get_compiler_flags()
        new = [f.replace("--enable-ldw-opt=false", "--enable-ldw-opt=true") for f in flags]
        if new != flags:
            set_compiler_flags(new)
    except Exception:
        pass


def run_cores(in_maps, **kwargs):
    from concourse.bass_utils import run_bass_kernel_spmd

    _enable_ldw_opt()
    nc = get_program()
    return run_bass_kernel_spmd(nc, in_maps, core_ids=list(range(NCORES)), **kwargs)


def assemble(res):
    outs = []
    for r in res.results:
        buf = np.asarray(r["out2"], dtype=np.float32)   # [T/4, 4, 8, BC]
        ys = buf.transpose(3, 0, 1, 2).reshape(BC, T, NY)
        outs.append(ys)
    return np.concatenate(outs, axis=0)


def kernel(useq, yz0, W1, b1, W2, b2, W3, b3):
    in_maps = _host_prep(useq, yz0, W1, b1, W2, b2, W3, b3)
    res = run_cores(in_maps)
    return assemble(res)


# revision 34
# speedup vs baseline: 1.4743x; 1.0028x over previous
"""Trainium2 Bass kernel for the NP/NY/NU RNN scan (nn_BlackBoxModel_24489903521937).

Model (per step t, batch row b):
    x_t   = [y_t, y_{t-4..t-1}, u_{t-4..t-1}, u_t]          (60)
    h1    = tanh(x_t @ W1 + b1)                              (128)
    h2    = tanh(h1 @ W2 + b2)                               (128)
    y_{t+1} = h2 @ W3 + b3                                   (8)
    output ys[:, t] = y_t

Strategy (pure data parallel, batch 4096 -> 8 cores x 512):
  * feature-major layout: features on SBUF partitions, batch on the free dim.
  * y-history lives in 4 ring slots of a [128, B] staging tile, one slot per
    32-partition strip.  The x @ W1 product is: one K=128 matmul against
    phase-permuted W1 blocks (C_p, p = t mod 4), one K=32 matmul against the
    raw 5-step u window (B rows, a fresh DMA'd tile per step so the u path
    has no write-after-read coupling to the staging tile and prefetches
    deep), and a composed (W3 @ A0) matmul from h2 directly, so the
    recurrent cycle is just tanh -> mm(W2) -> tanh -> mm(W3 A0).
  * CHUNKS=2 column chunks software-pipeline the recurrent chain: while
    chunk 0 is in an activation, chunk 1's matmul runs, keeping ScalarE
    (the bottleneck engine) nearly fully busy.  Each chunk owns private
    full PSUM banks (PSUM bank read/write collisions are fatal), including
    per-chunk mm3/staging-write chains so each chunk's y lands a full step
    before mmX(t+2) reads it.
  * outputs retire from the staging tile by raw feature-major DMA every 4
    steps; the host does the final [T,8,B] -> [B,T,8] transpose.  u-window
    DMAs ride the GpSimd engine queue so the flush DMAs' long semaphore
    waits on the Sync queue cannot head-of-line block them.
  * a warm-up burst of back-to-back matmuls at kernel start engages the PE
    HAM clock gate (K=8/8, 2.4 GHz); the tight steady-state schedule keeps
    it warm.
  * matmul operands are fp16 (1 cycle/row, fp32 PSUM accumulate); the
    5-step fading memory of the state keeps fp16 error flat (~6e-4).
"""

import numpy as np

NP_, NY, NU = 4, 8, 4
B, T, H = 4096, 256, 128
NCORES = 8
BC = B // NCORES  # 512 batch rows per core
CHUNKS = 2        # column chunks for the critical tanh/matmul cycle
CW = BC // CHUNKS
PF = 6            # u-window DMA prefetch depth (steps ahead)
NSLOT = 4         # y ring slots (one per 32-partition strip)
HEAT = 14         # warm-up matmuls at kernel start

_COMPILED = {}


def _build_program():
    import concourse.mybir as mybir
    import concourse.tile as tile
    from concourse import bacc

    f32 = mybir.dt.float32
    fh = mybir.dt.float16
    Tanh = mybir.ActivationFunctionType.Tanh

    nc = bacc.Bacc("TRN2", target_bir_lowering=False, debug=False)

    d_stag0 = nc.dram_tensor("stag0", [128, BC], fh, kind="ExternalInput")
    d_uwin = nc.dram_tensor("uwin", [T, 32, BC], fh, kind="ExternalInput")
    # 8 C matrices: [0..3] steady phases (t % 4), [4..7] boot steps t=0..3
    d_cmats = nc.dram_tensor("cmats", [128, 8 * 128], fh, kind="ExternalInput")
    d_bmat = nc.dram_tensor("bmat", [32, 128], fh, kind="ExternalInput")
    d_w2 = nc.dram_tensor("w2", [128, 128], fh, kind="ExternalInput")
    d_wc = nc.dram_tensor("wc", [128, 128], fh, kind="ExternalInput")
    d_w3 = nc.dram_tensor("w3", [128, 8], fh, kind="ExternalInput")
    d_b1 = nc.dram_tensor("b1v", [128, 1], f32, kind="ExternalInput")
    d_b1b = nc.dram_tensor("b1b", [128, 1], f32, kind="ExternalInput")
    d_b2 = nc.dram_tensor("b2v", [128, 1], f32, kind="ExternalInput")
    d_b3 = nc.dram_tensor("b3v", [8, 1], f32, kind="ExternalInput")
    d_out2 = nc.dram_tensor("out2", [T // 4, 4, 8, BC], fh, kind="ExternalOutput")

    with tile.TileContext(nc) as tc:
        with (
            tc.tile_pool(name="const", bufs=1) as cpool,
            tc.tile_pool(name="stagp", bufs=1) as spool,
            tc.tile_pool(name="upool", bufs=8) as upool,
            tc.tile_pool(name="hpool", bufs=3) as hpool,
            tc.tile_pool(name="ph1a", bufs=2, space="PSUM") as ph1a,
            tc.tile_pool(name="ph1b", bufs=2, space="PSUM") as ph1b,
            tc.tile_pool(name="ph2a", bufs=1, space="PSUM") as ph2a,
            tc.tile_pool(name="ph2b", bufs=1, space="PSUM") as ph2b,
            tc.tile_pool(name="pypa", bufs=1, space="PSUM") as pypa,
            tc.tile_pool(name="pypb", bufs=1, space="PSUM") as pypb,
        ):
            t_cm = cpool.tile_from(d_cmats[:])

            # --- PE warm-up: dense back-to-back matmuls engage the HAM
            #     clock gate (K=8/8, 2.4 GHz) and cover the initial DMAs.
            #     Reuses the pypa bank (same tag): strictly PE-ordered before
            #     any step-0 use, so no PSUM collision is possible. ---
            pyp_pools = [pypa, pypb]
            heat = pypa.tile([128, BC], f32, name="heat", tag="yp")
            for _ in range(HEAT):
                nc.tensor.matmul(heat[:, :], t_cm[:, 0:128], t_cm[:, 0:BC])

            t_bm = cpool.tile_from(d_bmat[:])
            t_w2 = cpool.tile_from(d_w2[:])
            t_wc = cpool.tile_from(d_wc[:])
            t_w3 = cpool.tile_from(d_w3[:])
            t_b1 = cpool.tile_from(d_b1[:])
            t_b1b = cpool.tile_from(d_b1b[:])
            t_b2 = cpool.tile_from(d_b2[:])
            t_b3 = cpool.tile_from(d_b3[:])

            stag = spool.tile([128, BC], fh, name="stag")
            nc.sync.dma_start(stag[:], d_stag0[:])

            def cmat(i):
                return t_cm[:, 128 * i:128 * i + 128]

            ph1_pools = [ph1a, ph1b]
            ph2_pools = [ph2a, ph2b]
            utiles = {}

            def prefetch_u(tt):
                # GpSimd-engine queue: keeps these triggers off the Sync
                # queue, where the flush DMAs' long semaphore waits would
                # block them (head-of-line) and land the u windows late.
                ut = upool.tile([32, BC], fh, name="uw", tag="uw")
                nc.gpsimd.dma_start(ut[:], d_uwin[tt])
                utiles[tt] = ut

            for tt in range(PF):
                prefetch_u(tt)

            def emit_mmx(tt, ph1_t):
                """y-history + u-window matmuls for step tt (chunked).

                Opens the ph1(tt) accumulation group; mmC of step tt-1
                (emitted later, executed later) closes it.
                """
                cidx = 4 + tt if tt < 4 else tt % NSLOT
                ut = utiles.pop(tt)
                for c in range(CHUNKS):
                    cs = slice(c * CW, (c + 1) * CW)
                    nc.tensor.matmul(
                        ph1_t[c][:, 0:CW],
                        cmat(cidx),
                        stag[:, cs],
                        start=True, stop=False, skip_group_check=True,
                    )
                    nc.tensor.matmul(
                        ph1_t[c][:, 0:CW],
                        t_bm[:, :],
                        ut[:, cs],
                        start=False, stop=(tt == 0), skip_group_check=True,
                    )

            def flush(t0):
                """Export y_{t0..t0+3} (all 4 slots) feature-major to DRAM;
                the host transposes to batch-major at the end.

                Emitted at step t0+3 BEFORE that step's staging write, so slot
                (t0+4)%4 still holds y_{t0}.
                """
                for s in range(4):
                    nc.sync.dma_start(
                        d_out2[t0 // 4, s], stag[32 * s:32 * s + 8, :]
                    )

            ph1_cur = [ph1_pools[c].tile([128, BC], f32, name="h1p", tag="h1p")
                       for c in range(CHUNKS)]
            emit_mmx(0, ph1_cur)

            for t in range(T):
                # --- tanh1 ---
                h1_t = hpool.tile([128, BC], fh, name="h1", tag="h1")
                bias1 = t_b1b if t == 0 else t_b1
                for c in range(CHUNKS):
                    cs = slice(c * CW, (c + 1) * CW)
                    nc.scalar.activation(
                        h1_t[:, cs], ph1_cur[c][:, 0:CW], Tanh, bias=bias1[:, 0:1]
                    )

                # --- pre-issue next step's x-side matmuls (fill the PE while
                #     the activations run; must precede this step's staging
                #     write so the stale y_{t-3} read stays dependency-free) ---
                ph1_next = None
                if t + 1 < T:
                    ph1_next = [
                        ph1_pools[c].tile([128, BC], f32, name="h1p", tag="h1p")
                        for c in range(CHUNKS)
                    ]
                    emit_mmx(t + 1, ph1_next)

                # --- mm2 ---
                ph2_t = [ph2_pools[c].tile([128, BC], f32, name="h2p", tag="h2p")
                         for c in range(CHUNKS)]
                with tc.high_priority():
                    for c in range(CHUNKS):
                        cs = slice(c * CW, (c + 1) * CW)
                        nc.tensor.matmul(
                            ph2_t[c][:, 0:CW],
                            t_w2[:, :],
                            h1_t[:, cs],
                        )

                # --- tanh2 ---
                h2_t = hpool.tile([128, BC], fh, name="h2", tag="h2")
                for c in range(CHUNKS):
                    cs = slice(c * CW, (c + 1) * CW)
                    nc.scalar.activation(
                        h2_t[:, cs], ph2_t[c][:, 0:CW], Tanh, bias=t_b2[:, 0:1]
                    )

                # --- mmC: ph1(t+1) += (W3 A0)^T h2_t  (closes the group) ---
                if t + 1 < T:
                    with tc.high_priority():
                        for c in range(CHUNKS):
                            cs = slice(c * CW, (c + 1) * CW)
                            nc.tensor.matmul(
                                ph1_next[c][:, 0:CW],
                                t_wc[:, :],
                                h2_t[:, cs],
                                start=False, stop=True,
                                skip_group_check=True,
                            )

                # --- output flush (before this step's staging write) ---
                if t % 4 == 3:
                    flush(t - 3)

                # --- mm3 + staging write (y_{t+1} = W3^T h2 + b3), chunked
                #     so each chunk's y lands a full step before mmX(t+2)
                #     reads it and stays off the recurrent critical path ---
                if t < T - 1:
                    s_new = (t + 1) % NSLOT
                    for c in range(CHUNKS):
                        cs = slice(c * CW, (c + 1) * CW)
                        pyp_t = pyp_pools[c].tile(
                            [128, BC], f32, name="yp", tag="yp")
                        nc.tensor.matmul(
                            pyp_t[0:8, 0:CW], t_w3[:, :], h2_t[:, cs]
                        )
                        nc.vector.tensor_scalar_add(
                            stag[32 * s_new:32 * s_new + 8, cs],
                            pyp_t[0:8, 0:CW], t_b3[:, 0:1]
                        )

                if t + PF < T:
                    prefetch_u(t + PF)

                ph1_cur = ph1_next

    nc.compile()
    return nc


def _host_prep(useq, yz0, W1, b1, W2, b2, W3, b3):
    """Build the per-core input maps (all host-side numpy)."""
    useq = np.ascontiguousarray(useq, dtype=np.float32)
    yz0 = np.ascontiguousarray(yz0, dtype=np.float32)
    W1 = np.asarray(W1, dtype=np.float32)
    W2 = np.ascontiguousarray(W2, dtype=np.float32)
    W3 = np.ascontiguousarray(W3, dtype=np.float32)
    b1 = np.asarray(b1, dtype=np.float32)
    b2 = np.asarray(b2, dtype=np.float32)
    b3 = np.asarray(b3, dtype=np.float32)

    A = {0: W1[0:8], 4: W1[8:16], 3: W1[16:24], 2: W1[24:32], 1: W1[32:40]}
    Bstack = W1[40:60]  # u_{t-4..t} stacked chronologically

    # staging rows: slot s -> [32s, 32s+8) holds the y ring;
    #               boot block s -> [32s+8, 32s+16) holds y_{-(s+1)}
    cmats = np.zeros((8, 128, 128), dtype=np.float32)
    for p in range(NSLOT):  # steady phases, t >= 4: every slot one A_k
        for s in range(NSLOT):
            k = ((p - s - 1) % 4) + 1
            cmats[p, 32 * s:32 * s + 8] = A[k]
    for tt in range(4):  # boot steps t=0..3
        cb = cmats[4 + tt]
        for k in range(1, 5):
            if tt - k >= 0:
                s = (tt - k) % 4
                cb[32 * s:32 * s + 8] += A[k]
            else:
                s = k - tt - 1
                cb[32 * s + 8:32 * s + 16] += A[k]
        if tt == 0:
            cb[0:8] += A[0]  # slot 0 carries y_0 directly at t=0
    cmats2d = np.ascontiguousarray(
        cmats.transpose(1, 0, 2).reshape(128, 8 * 128)
    )

    bmat = np.zeros((32, 128), dtype=np.float32)
    bmat[0:20] = Bstack

    WC = np.ascontiguousarray(W3 @ A[0])          # [128, 128]
    b1_eff = (b1 + A[0].T @ b3).reshape(128, 1)   # mmC path lacks A0^T b3
    b1_boot = b1.reshape(128, 1)
    b2v = b2.reshape(128, 1)
    b3v = b3.reshape(8, 1)

    cmats_fh = cmats2d.astype(np.float16)
    bmat_fh = bmat.astype(np.float16)
    w2_fh = W2.astype(np.float16)
    wc_fh = WC.astype(np.float16)
    w3_fh = W3.astype(np.float16)

    in_maps = []
    for c in range(NCORES):
        bs = slice(c * BC, (c + 1) * BC)
        u_c = useq[bs]      # [BC, T, 4]
        yz_c = yz0[bs]      # [BC, 56]

        stag0 = np.zeros((128, BC), dtype=np.float32)
        stag0[0:8] = yz_c[:, 0:8].T               # slot 0 = y_0
        for s in range(4):                         # boot blocks y_{-(s+1)}
            blk = yz_c[:, 8 + 8 * (3 - s):16 + 8 * (3 - s)]  # ypseq newest last
            stag0[32 * s + 8:32 * s + 16] = blk.T

        # sliding u-windows for the K=32 u matmul (rows 20..31 zero)
        uhist = yz_c[:, 40:56].reshape(BC, 4, 4)          # u_{-4..-1}
        uext = np.concatenate([uhist, u_c], axis=1)       # [BC, T+4, 4]
        sw = np.lib.stride_tricks.sliding_window_view(uext, 5, axis=1)
        # sw: [BC, T, 4, 5] -> uwin [T, 20, BC] (chronological rows)
        uwin = np.zeros((T, 32, BC), dtype=np.float16)
        uwin[:, 0:20] = sw.transpose(1, 3, 2, 0).reshape(T, 20, BC)

        in_maps.append({
            "stag0": stag0.astype(np.float16),
            "uwin": uwin,
            "cmats": cmats_fh,
            "bmat": bmat_fh,
            "w2": w2_fh,
            "wc": wc_fh,
            "w3": w3_fh,
            "b1v": np.ascontiguousarray(b1_eff),
            "b1b": np.ascontiguousarray(b1_boot),
            "b2v": np.ascontiguousarray(b2v),
            "b3v": np.ascontiguousarray(b3v),
        })
    return in_maps


def get_program():
    if "nc" not in _COMPILED:
        _enable_ldw_opt()
        _COMPILED["nc"] = _build_program()
    return _COMPILED["nc"]


def _enable_ldw_opt():
    """Allow walrus to double-buffer LDWEIGHTS (background weight loads).

    The environment default is --enable-ldw-opt=false, which serializes
    every LDWEIGHTS behind the previous matmul's drain; with ~9 weight
    switches per RNN step that costs ~2x on the tensor engine.
    """
    try:
        from concourse.compiler_utils import get_compiler_flags, set_compiler_flags

        flags = get_compiler_flags()
        new = [f.replace("--enable-ldw-opt=false", "--enable-ldw-opt=true") for f in flags]
        if new != flags:
            set_compiler_flags(new)
    except Exception:
        pass


def run_cores(in_maps, **kwargs):
    from concourse.bass_utils import run_bass_kernel_spmd

    _enable_ldw_opt()
    nc = get_program()
    return run_bass_kernel_spmd(nc, in_maps, core_ids=list(range(NCORES)), **kwargs)


def assemble(res):
    outs = []
    for r in res.results:
        buf = np.asarray(r["out2"], dtype=np.float32)   # [T/4, 4, 8, BC]
        ys = buf.transpose(3, 0, 1, 2).reshape(BC, T, NY)
        outs.append(ys)
    return np.concatenate(outs, axis=0)


def kernel(useq, yz0, W1, b1, W2, b2, W3, b3):
    in_maps = _host_prep(useq, yz0, W1, b1, W2, b2, W3, b3)
    res = run_cores(in_maps)
    return assemble(res)
